# revision 46
# baseline (speedup 1.0000x reference)
"""Bahdanau attention Trainium2 kernel.

Full-input contract: kernel(**inputs) takes the complete (unsharded) numpy
inputs and returns (context, alphas, new_coverage) matching reference().

Strategy: pure data-parallel over batch (32 batches -> 4 per NeuronCore x 8).
Per core, one streaming pass over `value`:
  feat = value @ Wv + q + pc*Wc  accumulated in PSUM (token-major tiles),
  tanh on ScalarE, scores via fused DVE multiply-reduce against broadcast We,
  softmax without max-subtraction (scores are bounded, exp is safe),
  context recovered without storing v:  since sum(alphas) == 1,
    context = sum_s alphas_s * feat[s,:] - q - Wc * sum_s(alphas_s * pc_s).
"""
import os
import sys
import time

sys.path.insert(0, "/opt/trn_rl_repo")

import numpy as np
import ml_dtypes

import concourse.bass as bass
import concourse.tile as tile
from concourse import bacc, mybir
from concourse import bass_utils

F32 = mybir.dt.float32
BF16 = mybir.dt.bfloat16
F32R = mybir.dt.float32r
AF = mybir.ActivationFunctionType
ALU = mybir.AluOpType

H = 1024          # hidden dim
S = 2048          # sequence length
B = 32            # global batch
NCORES = 8
BLOC = B // NCORES  # 4 batches per core
KC = 16           # contraction chunks (2H / 128)
NTT = S // 128    # 16 token tiles of 128 per batch
TOK = BLOC * S    # 8192 tokens per core

# dtype configuration for the heavy matmuls / retained feat tiles
CFG = dict(
    mm="bf16",     # "bf16" | "f32r" | "f32"   main matmul operand dtype
    feat="bf16",   # "bf16" | "f32"            retained feat / tanh dtype
    gt=512,        # tokens per value DMA group
    tanh_bufs=3,
    scr_bufs=2,
    ablate="",     # comma list: noctx,noepi,gN (limit value groups) — debug
)


def _abl(flag):
    return flag in CFG["ablate"].split(",")


def _abl_groups():
    for f in CFG["ablate"].split(","):
        if f.startswith("g") and f[1:].isdigit():
            return int(f[1:])
    return 10**9


def _mm_dt():
    return {"bf16": BF16, "f32": F32, "f32r": F32R}[CFG["mm"]]


def _mm_np():
    return ml_dtypes.bfloat16 if CFG["mm"] == "bf16" else np.float32


def _feat_dt():
    return BF16 if CFG["feat"] == "bf16" else F32


def _feat_np():
    return ml_dtypes.bfloat16 if CFG["feat"] == "bf16" else np.float32


def _mmv(ap):
    return ap


def _featv(ap):
    return ap


def _ctx_dt():
    """dtype for feat/a4/pcTf (context matmul operands): ride the fast f32r
    path when feat is f32 and the main matmul is f32r."""
    fdt = _feat_dt()
    if fdt == F32 and CFG["mm"] == "f32r":
        return F32R
    return fdt


def build_kernel(repeat=1):
    """Build the per-core Bass program. `repeat` re-runs the whole pipeline
    (same data) for wall-clock timing; the graded path uses repeat=1."""
    nc = bacc.Bacc("TRN2", target_bir_lowering=False, debug=False,
                   num_devices=NCORES)
    mdt, fdt = _mm_dt(), _feat_dt()
    cdt = _ctx_dt()
    gt = CFG["gt"]

    valT = nc.dram_tensor("valT", [KC, 128, TOK], mdt, kind="ExternalInput").ap()
    wv = nc.dram_tensor("wv", [KC, 128, H], mdt, kind="ExternalInput").ap()
    wq = nc.dram_tensor("wq", [KC, 128, H], mdt, kind="ExternalInput").ap()
    qT = nc.dram_tensor("qT", [KC, 128, BLOC], mdt, kind="ExternalInput").ap()
    pcT = nc.dram_tensor("pcT", [128, BLOC * NTT], F32, kind="ExternalInput").ap()
    pcTf = nc.dram_tensor("pcTf", [128, BLOC * NTT], cdt, kind="ExternalInput").ap()
    pc_row = nc.dram_tensor("pc_row", [1, TOK], mdt, kind="ExternalInput").ap()
    wc_row = nc.dram_tensor("wc_row", [1, H], mdt, kind="ExternalInput").ap()
    mpen = nc.dram_tensor("mpen", [128, BLOC * NTT], F32, kind="ExternalInput").ap()
    web = nc.dram_tensor("web", [128, H], fdt, kind="ExternalInput").ap()
    wc4n = nc.dram_tensor("wc4n", [BLOC, H], F32, kind="ExternalInput").ap()

    ctx_out = nc.dram_tensor("ctx", [BLOC, H], F32, kind="ExternalOutput").ap()
    alph_out = nc.dram_tensor("alph", [BLOC, S], F32, kind="ExternalOutput").ap()
    ncov_out = nc.dram_tensor("ncov", [BLOC, S], F32, kind="ExternalOutput").ap()

    with tile.TileContext(nc) as tc:
        with (
            tc.tile_pool(name="wvp", bufs=1) as wvp,
            tc.tile_pool(name="consts", bufs=1) as consts,
            tc.tile_pool(name="vg", bufs=2) as vgp,
            tc.tile_pool(name="featbuf", bufs=2 if CFG["feat"] == "bf16" else 1) as featp,
            tc.tile_pool(name="tanh", bufs=CFG["tanh_bufs"]) as tanhp,
            tc.tile_pool(name="scr", bufs=CFG["scr_bufs"]) as scrp,
            tc.tile_pool(name="small", bufs=2) as smallp,
            tc.tile_pool(name="featps", bufs=3, space="PSUM") as featps,
            tc.tile_pool(name="ctxps", bufs=1, space="PSUM") as ctxps,
            tc.tile_pool(name="spsum", bufs=2, space="PSUM") as spsum,
        ):
            # ---- constants / weights -------------------------------------
            wv_sb = wvp.tile([128, KC, H], mdt, name="wv_sb")
            nc.sync.dma_start(wv_sb[:], wv.transpose([1, 0, 2]))
            qT_sb = consts.tile([128, KC, BLOC], mdt, name="qT_sb")
            nc.sync.dma_start(qT_sb[:], qT.transpose([1, 0, 2]))
            pcT_sb = consts.tile([128, BLOC * NTT], F32, name="pcT_sb")
            nc.sync.dma_start(pcT_sb[:], pcT)
            pcTf_sb = consts.tile([128, BLOC * NTT], cdt, name="pcTf_sb")
            nc.sync.dma_start(pcTf_sb[:], pcTf)
            mpen_sb = consts.tile([128, BLOC * NTT], F32, name="mpen_sb")
            nc.sync.dma_start(mpen_sb[:], mpen)
            web_sb = consts.tile([128, H], fdt, name="web_sb")
            nc.sync.dma_start(web_sb[:], web)
            wc4n_sb = consts.tile([BLOC, H], F32, name="wc4n_sb")
            nc.sync.dma_start(wc4n_sb[:], wc4n)
            wcr_sb = consts.tile([1, H], mdt, name="wcr_sb")
            nc.sync.dma_start(wcr_sb[:], wc_row)

            ones_f = consts.tile([128, 128], F32, name="ones_f")
            nc.vector.memset(ones_f[:], 1.0)
            zeros_f = consts.tile([128, NTT * BLOC], F32, name="zeros_f")
            nc.vector.memset(zeros_f[:], 0.0)
            if mdt == F32:
                ones_mm = ones_f
            else:
                ones_mm = consts.tile([128, 128], mdt, name="ones_mm")
                nc.vector.tensor_copy(ones_mm[:], ones_f[:])

            q_sb4 = consts.tile([BLOC, H], F32, name="q_sb4")
            if mdt != F32:
                q_rnd = consts.tile([BLOC, H], mdt, name="q_rnd")
                # f32 image of the rounded q actually added into feat; the
                # context correction must subtract exactly this value.
                q_used = consts.tile([BLOC, H], F32, name="q_used")
            else:
                q_used = q_sb4

            import contextlib
            loop_ctx = (tc.For_i(0, repeat, 1) if repeat > 1
                        else contextlib.nullcontext())
            with loop_ctx:
              for rep in range(1):
                u = "r0"
                # ---- q = query @ Wq  -> q_sb4 [4, H] -------------------
                # Wq streams through the featbuf pool slots in two halves.
                qps = [spsum.tile([BLOC, 512], F32, tag="sp", name=f"qp{h}_{u}")
                       for h in range(2)]
                for hw in range(2):
                    wq_h = featp.tile([128, KC // 2, H], mdt, tag="featbuf",
                                      name=f"wq_h{hw}_{u}")
                    nc.sync.dma_start(
                        wq_h[:],
                        wq[hw * (KC // 2):(hw + 1) * (KC // 2)].transpose([1, 0, 2]))
                    for half in range(2):
                        for k8 in range(KC // 2):
                            kc = hw * (KC // 2) + k8
                            nc.tensor.matmul(
                                qps[half][:],
                                _mmv(qT_sb[:, kc, :]),
                                _mmv(wq_h[:, k8, half * 512:(half + 1) * 512]),
                                start=(kc == 0), stop=(kc == KC - 1),
                            )
                for half in range(2):
                    nc.vector.tensor_copy(
                        q_sb4[:, half * 512:(half + 1) * 512], qps[half][:])
                if mdt != F32:
                    nc.vector.tensor_copy(q_rnd[:], q_sb4[:])
                    nc.vector.tensor_copy(q_used[:], q_rnd[:])
                    qsrc = q_rnd
                else:
                    qsrc = q_sb4

                ctx_psum = ctxps.tile([BLOC, H], F32, tag="ctx", name=f"ctxps_{u}")
                apc_sb4 = consts.tile([BLOC, 1], F32, name=f"apc_sb4_{u}",
                                      uniquify=True)
                apc_row = consts.tile([1, BLOC], F32, name=f"apc_row_{u}",
                                      uniquify=True)

                # ---- main streaming pass ---------------------------------
                glim = _abl_groups()
                gcount = 0
                for b in range(BLOC):
                    featbuf = featp.tile([128, NTT * H], cdt, tag="featbuf",
                                         name=f"featbuf_b{b}_{u}")
                    scores = smallp.tile([128, NTT], F32, tag="scores",
                                         name=f"scores_b{b}_{u}")
                    pcr_b = smallp.tile([1, S], mdt, tag="rowpc",
                                        bufs=2 if CFG["feat"] == "bf16" else 1,
                                        name=f"pcr_b{b}_{u}")
                    nc.sync.dma_start(pcr_b[:], pc_row[0:1, b * S:(b + 1) * S])
                    q_b = smallp.tile([1, H], mdt, tag="qb", bufs=2,
                                      name=f"q_b{b}_{u}")
                    nc.sync.dma_start(q_b[:], qsrc[b:b + 1, :])
                    for g in range(S // gt):
                        gcount += 1
                        if gcount > glim:
                            continue
                        vg = vgp.tile([128, KC, gt], mdt, tag="vg",
                                      name=f"vg_b{b}g{g}_{u}")
                        t0 = b * S + g * gt
                        nc.sync.dma_start(
                            vg[:], valT[:, :, t0:t0 + gt].transpose([1, 0, 2]))
                        for sub in range(gt // 128):
                            tt = g * (gt // 128) + sub
                            pr = scrp.tile([128, 2], F32, tag="pr",
                                           name=f"pr_b{b}t{tt}_{u}")
                            fslice = featbuf[:, tt * H:(tt + 1) * H]
                            for half in range(2):
                                fp = featps.tile([128, 512], F32, tag="featps",
                                                 name=f"fp_b{b}t{tt}h{half}_{u}")
                                for kc in range(KC):
                                    nc.tensor.matmul(
                                        fp[:],
                                        _mmv(vg[:, kc, sub * 128:(sub + 1) * 128]),
                                        _mmv(wv_sb[:, kc, half * 512:(half + 1) * 512]),
                                        start=(kc == 0), stop=False,
                                    )
                                # + q[b,:] broadcast over tokens (K=1)
                                nc.tensor.matmul(
                                    fp[:],
                                    _mmv(ones_mm[0:1, 0:128]),
                                    _mmv(q_b[0:1, half * 512:(half + 1) * 512]),
                                    start=False, stop=False,
                                )
                                # + pc[s] * Wc[h] rank-1 coverage term (K=1)
                                nc.tensor.matmul(
                                    fp[:],
                                    _mmv(pcr_b[0:1, tt * 128:(tt + 1) * 128]),
                                    _mmv(wcr_sb[0:1, half * 512:(half + 1) * 512]),
                                    start=False, stop=True,
                                )
                                # spill feat PSUM -> SBUF
                                nc.vector.tensor_copy(
                                    fslice[:, half * 512:(half + 1) * 512], fp[:])
                                th = tanhp.tile([128, 512], fdt, tag="tanh",
                                                name=f"th_b{b}t{tt}h{half}_{u}")
                                nc.scalar.activation(th[:], fp[:], AF.Tanh)
                                scr = scrp.tile([128, 512], fdt, tag="scr",
                                                name=f"scr_b{b}t{tt}h{half}_{u}")
                                nc.vector.tensor_mul(
                                    scr[:], th[:],
                                    web_sb[:, half * 512:(half + 1) * 512])
                                nc.vector.tensor_reduce(
                                    pr[:, half:half + 1], scr[:],
                                    mybir.AxisListType.X, ALU.add)
                            nc.vector.tensor_add(scores[:, tt:tt + 1],
                                                 pr[:, 0:1], pr[:, 1:2])

                    # ---- batch epilogue: softmax + context ---------------
                    if _abl("noepi"):
                        continue
                    sm = smallp.tile([128, NTT], F32, tag="sm", name=f"sm_b{b}_{u}")
                    nc.vector.tensor_add(sm[:], scores[:],
                                         mpen_sb[:, b * NTT:(b + 1) * NTT])
                    p_mat = smallp.tile([128, NTT], F32, tag="pmat",
                                        name=f"p_b{b}_{u}")
                    sumcol = smallp.tile([128, 1], F32, tag="sumcol",
                                         name=f"sumcol_b{b}_{u}")
                    nc.scalar.activation(p_mat[:], sm[:], AF.Exp)
                    nc.vector.tensor_reduce(sumcol[:], p_mat[:],
                                            mybir.AxisListType.X, ALU.add)
                    dp = spsum.tile([1, 1], F32, tag="sp", name=f"dp_b{b}_{u}")
                    nc.tensor.matmul(dp[:], ones_f[:, 0:1], sumcol[:],
                                     start=True, stop=True)
                    inv_sb = smallp.tile([1, 1], F32, tag="inv",
                                         name=f"inv_b{b}_{u}")
                    nc.vector.reciprocal(inv_sb[:], dp[:])
                    ib = spsum.tile([128, 1], F32, tag="sp", name=f"ib_b{b}_{u}")
                    nc.tensor.matmul(ib[:], ones_f[0:1, 0:128], inv_sb[:],
                                     start=True, stop=True)
                    alphas = smallp.tile([128, NTT], F32, tag="alphas",
                                         name=f"alphas_b{b}_{u}")
                    nc.vector.tensor_scalar_mul(alphas[:], p_mat[:], ib[:])
                    cov = smallp.tile([128, NTT], F32, tag="cov",
                                      name=f"cov_b{b}_{u}")
                    nc.vector.tensor_add(cov[:], alphas[:],
                                         pcT_sb[:, b * NTT:(b + 1) * NTT])
                    nc.sync.dma_start(
                        alph_out[b].rearrange("(c p) -> p c", p=128), alphas[:])
                    nc.sync.dma_start(
                        ncov_out[b].rearrange("(c p) -> p c", p=128), cov[:])

                    if _abl("noctx"):
                        continue
                    # apc[b] = sum_s alphas * pc  (DVE + f32 partition-sum)
                    am = smallp.tile([128, NTT], F32, tag="am",
                                     name=f"am_b{b}_{u}")
                    nc.vector.tensor_mul(am[:], alphas[:],
                                         pcT_sb[:, b * NTT:(b + 1) * NTT])
                    red = smallp.tile([128, 1], F32, tag="apcred",
                                      name=f"red_b{b}_{u}")
                    nc.vector.tensor_reduce(red[:], am[:],
                                            mybir.AxisListType.X, ALU.add)
                    apc_ps = spsum.tile([1, 1], F32, tag="sp",
                                        name=f"apcps_b{b}_{u}")
                    nc.tensor.matmul(apc_ps[:], ones_f[:, 0:1], red[:],
                                     start=True, stop=True)
                    nc.vector.tensor_copy(apc_row[0:1, b:b + 1], apc_ps[:])

                    a4 = smallp.tile([128, NTT, BLOC], cdt, tag="a4",
                                     name=f"a4_b{b}_{u}")
                    if cdt == F32R:
                        nc.vector.tensor_copy(
                            a4[:].rearrange("p a b -> p (a b)"), zeros_f[:])
                    else:
                        nc.vector.memset(a4[:], 0.0)
                    nc.vector.tensor_copy(a4[:, :, b], alphas[:])
                    for tt in range(NTT):
                        st = (b == 0 and tt == 0)
                        sp = (b == BLOC - 1 and tt == NTT - 1)
                        for half in range(2):
                            nc.tensor.matmul(
                                ctx_psum[:, half * 512:(half + 1) * 512],
                                _featv(a4[:, tt, :]),
                                _featv(featbuf[:, tt * H + half * 512:
                                               tt * H + (half + 1) * 512]),
                                start=st, stop=sp,
                            )

                # ---- final context correction ----------------------------
                if _abl("noctx") or _abl("noepi"):
                    zc = smallp.tile([BLOC, H], F32, tag="ctxsb", bufs=1,
                                     name=f"zc_{u}")
                    nc.vector.memset(zc[:], 0.0)
                    nc.sync.dma_start(ctx_out[:], zc[:])
                    continue
                nc.sync.dma_start(apc_sb4[:], apc_row[:])
                tmp_sb = smallp.tile([BLOC, H], F32, tag="tmp", bufs=1,
                                     name=f"tmp_{u}")
                nc.vector.tensor_sub(tmp_sb[:], ctx_psum[:], q_used[:])
                # q_sb4 is dead after the subtraction above; reuse as scratch
                nc.vector.tensor_scalar_mul(q_sb4[:], wc4n_sb[:],
                                            apc_sb4[:, 0:1])
                ctx_sb = smallp.tile([BLOC, H], F32, tag="ctxsb", bufs=1,
                                     name=f"ctxsb_{u}")
                nc.vector.tensor_add(ctx_sb[:], tmp_sb[:], q_sb4[:])
                nc.sync.dma_start(ctx_out[:], ctx_sb[:])

    nc.compile()
    return nc


# ---------------------------------------------------------------------------
# host-side input prep
# ---------------------------------------------------------------------------

def prep_in_maps(value, query, padding_mask, pre_coverage, Wv, Wq, We, Wc):
    mmnp, fnp = _mm_np(), _feat_np()
    wv_h = np.ascontiguousarray(Wv.reshape(KC, 128, H)).astype(mmnp)
    wq_h = np.ascontiguousarray(Wq.reshape(KC, 128, H)).astype(mmnp)
    web_h = np.ascontiguousarray(np.broadcast_to(We, (128, H))).astype(fnp)
    wc_row_h = Wc.reshape(1, H).astype(mmnp)
    # the context correction must subtract Wc exactly as rounded into feat
    wc_used = wc_row_h.astype(np.float32)
    wc4n_h = np.ascontiguousarray(np.broadcast_to(-wc_used, (BLOC, H)))

    in_maps = []
    for c in range(NCORES):
        sl = slice(c * BLOC, (c + 1) * BLOC)
        val_c = value[sl]                      # [4, S, 2H]
        valT = val_c.transpose(2, 0, 1).reshape(KC, 128, TOK).astype(mmnp)
        qT = query[sl].T.reshape(KC, 128, BLOC).astype(mmnp)
        pcT = np.ascontiguousarray(
            pre_coverage[sl].reshape(BLOC, NTT, 128).transpose(2, 0, 1)
        ).reshape(128, BLOC * NTT).astype(np.float32)
        mp = np.ascontiguousarray(
            padding_mask[sl].reshape(BLOC, NTT, 128).transpose(2, 0, 1)
        ).reshape(128, BLOC * NTT).astype(np.float32) * np.float32(-1e30)
        in_maps.append({
            "valT": valT, "wv": wv_h, "wq": wq_h, "qT": qT,
            "pcT": pcT, "pcTf": pcT.astype(fnp), "mpen": mp,
            "pc_row": pre_coverage[sl].reshape(1, TOK).astype(mmnp),
            "wc_row": wc_row_h,
            "web": web_h, "wc4n": wc4n_h,
        })
    return in_maps


def assemble_outputs(results):
    ctx = np.stack([results[c]["ctx"] for c in range(NCORES)])    # [8,4,H]
    alph = np.stack([results[c]["alph"] for c in range(NCORES)])  # [8,4,S]
    ncov = np.stack([results[c]["ncov"] for c in range(NCORES)])
    context = ctx.reshape(B, H)[:, None, :].astype(np.float32)
    alphas = alph.reshape(B, S).astype(np.float32)
    new_coverage = ncov.reshape(B, S).astype(np.float32)
    return context, alphas, new_coverage


_CACHED_NC = None


def kernel(value, query, padding_mask, pre_coverage, Wv, Wq, We, Wc):
    global _CACHED_NC
    value = np.asarray(value)
    query = np.asarray(query)
    padding_mask = np.asarray(padding_mask)
    pre_coverage = np.asarray(pre_coverage)
    Wv, Wq = np.asarray(Wv), np.asarray(Wq)
    We, Wc = np.asarray(We), np.asarray(Wc)

    if _CACHED_NC is None:
        _CACHED_NC = build_kernel()
    nc = _CACHED_NC
    in_maps = prep_in_maps(value, query, padding_mask, pre_coverage,
                           Wv, Wq, We, Wc)
    res = bass_utils.run_bass_kernel_spmd(nc, in_maps,
                                          core_ids=list(range(NCORES)))
    return assemble_outputs(res.results)


# revision 49
# speedup vs baseline: 1.0662x; 1.0662x over previous
"""Bahdanau attention Trainium2 kernel.

Full-input contract: kernel(**inputs) takes the complete (unsharded) numpy
inputs and returns (context, alphas, new_coverage) matching reference().

Strategy: pure data-parallel over batch (32 batches -> 4 per NeuronCore x 8).
Per core, one streaming pass over `value`:
  feat = value @ Wv + q + pc*Wc  accumulated in PSUM (token-major tiles),
  tanh on ScalarE, scores via fused DVE multiply-reduce against broadcast We,
  softmax without max-subtraction (scores are bounded, exp is safe),
  context recovered without storing v:  since sum(alphas) == 1,
    context = sum_s alphas_s * feat[s,:] - q - Wc * sum_s(alphas_s * pc_s).
"""
import os
import sys
import time

sys.path.insert(0, "/opt/trn_rl_repo")

import numpy as np
import ml_dtypes

import concourse.bass as bass
import concourse.tile as tile
from concourse import bacc, mybir
from concourse import bass_utils

F32 = mybir.dt.float32
BF16 = mybir.dt.bfloat16
F32R = mybir.dt.float32r
AF = mybir.ActivationFunctionType
ALU = mybir.AluOpType

H = 1024          # hidden dim
S = 2048          # sequence length
B = 32            # global batch
NCORES = 8
BLOC = B // NCORES  # 4 batches per core
KC = 16           # contraction chunks (2H / 128)
NTT = S // 128    # 16 token tiles of 128 per batch
TOK = BLOC * S    # 8192 tokens per core

# dtype configuration for the heavy matmuls / retained feat tiles
CFG = dict(
    mm="bf16",     # "bf16" | "f32r" | "f32"   main matmul operand dtype
    feat="bf16",   # "bf16" | "f32"            retained feat / tanh dtype
    gt=512,        # tokens per value DMA group
    tanh_bufs=3,
    scr_bufs=2,
    ablate="",     # comma list: noctx,noepi,gN (limit value groups) — debug
)


def _abl(flag):
    return flag in CFG["ablate"].split(",")


def _abl_groups():
    for f in CFG["ablate"].split(","):
        if f.startswith("g") and f[1:].isdigit():
            return int(f[1:])
    return 10**9


def _mm_dt():
    return {"bf16": BF16, "f32": F32, "f32r": F32R}[CFG["mm"]]


def _mm_np():
    return ml_dtypes.bfloat16 if CFG["mm"] == "bf16" else np.float32


def _feat_dt():
    return BF16 if CFG["feat"] == "bf16" else F32


def _feat_np():
    return ml_dtypes.bfloat16 if CFG["feat"] == "bf16" else np.float32


def _mmv(ap):
    return ap


def _featv(ap):
    return ap


def _round_f32r(a):
    """Round fp32 array to the f32r grid (11 mantissa bits, RNE)."""
    u = np.ascontiguousarray(a, dtype=np.float32).view(np.uint32)
    shift = 23 - 11
    lsb = (u >> shift) & np.uint32(1)
    r = (u + np.uint32((1 << (shift - 1)) - 1) + lsb) & np.uint32(
        0xFFFFFFFF ^ ((1 << shift) - 1))
    return r.view(np.float32).reshape(a.shape)


def _ctx_dt():
    """dtype for feat/a4/pcTf (context matmul operands): ride the fast f32r
    path when feat is f32 and the main matmul is f32r."""
    fdt = _feat_dt()
    if fdt == F32 and CFG["mm"] == "f32r":
        return F32R
    return fdt


def build_kernel(repeat=1):
    """Build the per-core Bass program. `repeat` re-runs the whole pipeline
    (same data) for wall-clock timing; the graded path uses repeat=1."""
    nc = bacc.Bacc("TRN2", target_bir_lowering=False, debug=False,
                   num_devices=NCORES)
    mdt, fdt = _mm_dt(), _feat_dt()
    cdt = _ctx_dt()
    gt = CFG["gt"]

    valT = nc.dram_tensor("valT", [KC, 128, TOK], mdt, kind="ExternalInput").ap()
    wv = nc.dram_tensor("wv", [KC, 128, H], mdt, kind="ExternalInput").ap()
    wq = nc.dram_tensor("wq", [KC, 128, H], mdt, kind="ExternalInput").ap()
    qT = nc.dram_tensor("qT", [KC, 128, BLOC], mdt, kind="ExternalInput").ap()
    pcT = nc.dram_tensor("pcT", [128, BLOC * NTT], F32, kind="ExternalInput").ap()
    pcTf = nc.dram_tensor("pcTf", [128, BLOC * NTT], cdt, kind="ExternalInput").ap()
    qpc2 = nc.dram_tensor("qpc2", [2, TOK], mdt, kind="ExternalInput").ap()
    wc_row = nc.dram_tensor("wc_row", [1, H], mdt, kind="ExternalInput").ap()
    mpen = nc.dram_tensor("mpen", [128, BLOC * NTT], F32, kind="ExternalInput").ap()
    web = nc.dram_tensor("web", [128, H], fdt, kind="ExternalInput").ap()
    wc4n = nc.dram_tensor("wc4n", [BLOC, H], F32, kind="ExternalInput").ap()

    ctx_out = nc.dram_tensor("ctx", [BLOC, H], F32, kind="ExternalOutput").ap()
    alph_out = nc.dram_tensor("alph", [BLOC, S], F32, kind="ExternalOutput").ap()
    ncov_out = nc.dram_tensor("ncov", [BLOC, S], F32, kind="ExternalOutput").ap()

    with tile.TileContext(nc) as tc:
        with (
            tc.tile_pool(name="wvp", bufs=1) as wvp,
            tc.tile_pool(name="consts", bufs=1) as consts,
            tc.tile_pool(name="vg", bufs=2) as vgp,
            tc.tile_pool(name="featbuf", bufs=2 if CFG["feat"] == "bf16" else 1) as featp,
            tc.tile_pool(name="tanh", bufs=CFG["tanh_bufs"]) as tanhp,
            tc.tile_pool(name="scr", bufs=CFG["scr_bufs"]) as scrp,
            tc.tile_pool(name="small", bufs=2) as smallp,
            tc.tile_pool(name="featps", bufs=3, space="PSUM") as featps,
            tc.tile_pool(name="ctxps", bufs=1, space="PSUM") as ctxps,
            tc.tile_pool(name="spsum", bufs=2, space="PSUM") as spsum,
        ):
            # ---- constants / weights -------------------------------------
            wv_sb = wvp.tile([128, KC, H], mdt, name="wv_sb")
            nc.sync.dma_start(wv_sb[:], wv.transpose([1, 0, 2]))
            qT_sb = consts.tile([128, KC, BLOC], mdt, name="qT_sb")
            nc.sync.dma_start(qT_sb[:], qT.transpose([1, 0, 2]))
            pcT_sb = consts.tile([128, BLOC * NTT], F32, name="pcT_sb")
            nc.sync.dma_start(pcT_sb[:], pcT)
            pcTf_sb = consts.tile([128, BLOC * NTT], cdt, name="pcTf_sb")
            nc.sync.dma_start(pcTf_sb[:], pcTf)
            mpen_sb = consts.tile([128, BLOC * NTT], F32, name="mpen_sb")
            nc.sync.dma_start(mpen_sb[:], mpen)
            web_sb = consts.tile([128, H], fdt, name="web_sb")
            nc.sync.dma_start(web_sb[:], web)
            wc4n_sb = consts.tile([BLOC, H], F32, name="wc4n_sb")
            nc.sync.dma_start(wc4n_sb[:], wc4n)

            ones_f = consts.tile([128, 128], F32, name="ones_f")
            nc.vector.memset(ones_f[:], 1.0)
            zeros_f = consts.tile([128, NTT * BLOC], F32, name="zeros_f")
            nc.vector.memset(zeros_f[:], 0.0)

            q_sb4 = consts.tile([BLOC, H], F32, name="q_sb4")
            if mdt != F32:
                q_rnd = consts.tile([BLOC, H], mdt, name="q_rnd")
                # f32 image of the rounded q actually added into feat; the
                # context correction must subtract exactly this value.
                q_used = consts.tile([BLOC, H], F32, name="q_used")
            else:
                q_used = q_sb4

            import contextlib
            loop_ctx = (tc.For_i(0, repeat, 1) if repeat > 1
                        else contextlib.nullcontext())
            with loop_ctx:
              for rep in range(1):
                u = "r0"
                # ---- q = query @ Wq  -> q_sb4 [4, H] -------------------
                # Wq streams through the featbuf pool slots in two halves.
                qps = [spsum.tile([BLOC, 512], F32, tag="sp", name=f"qp{h}_{u}")
                       for h in range(2)]
                for hw in range(2):
                    wq_h = featp.tile([128, KC // 2, H], mdt, tag="featbuf",
                                      name=f"wq_h{hw}_{u}")
                    nc.sync.dma_start(
                        wq_h[:],
                        wq[hw * (KC // 2):(hw + 1) * (KC // 2)].transpose([1, 0, 2]))
                    for half in range(2):
                        for k8 in range(KC // 2):
                            kc = hw * (KC // 2) + k8
                            nc.tensor.matmul(
                                qps[half][:],
                                _mmv(qT_sb[:, kc, :]),
                                _mmv(wq_h[:, k8, half * 512:(half + 1) * 512]),
                                start=(kc == 0), stop=(kc == KC - 1),
                            )
                for half in range(2):
                    nc.vector.tensor_copy(
                        q_sb4[:, half * 512:(half + 1) * 512], qps[half][:])
                if mdt != F32:
                    nc.vector.tensor_copy(q_rnd[:], q_sb4[:])
                    nc.vector.tensor_copy(q_used[:], q_rnd[:])
                    qsrc = q_rnd
                else:
                    qsrc = q_sb4

                ctx_psum = ctxps.tile([BLOC, H], F32, tag="ctx", name=f"ctxps_{u}")
                apc_sb4 = consts.tile([BLOC, 1], F32, name=f"apc_sb4_{u}",
                                      uniquify=True)
                apc_row = consts.tile([1, BLOC], F32, name=f"apc_row_{u}",
                                      uniquify=True)

                # ---- main streaming pass ---------------------------------
                glim = _abl_groups()
                gcount = 0
                for b in range(BLOC):
                    featbuf = featp.tile([128, NTT * H], cdt, tag="featbuf",
                                         name=f"featbuf_b{b}_{u}")
                    scores = smallp.tile([128, NTT], F32, tag="scores",
                                         name=f"scores_b{b}_{u}")
                    qpc2_b = smallp.tile([2, S], mdt, tag="rowpc",
                                         bufs=2 if CFG["feat"] == "bf16" else 1,
                                         name=f"qpc2_b{b}_{u}")
                    nc.sync.dma_start(qpc2_b[:], qpc2[:, b * S:(b + 1) * S])
                    qwc_b = smallp.tile([2, H], mdt, tag="qwc", bufs=2,
                                        name=f"qwc_b{b}_{u}")
                    nc.sync.dma_start(qwc_b[0:1, :], qsrc[b:b + 1, :])
                    nc.sync.dma_start(qwc_b[1:2, :], wc_row)
                    for g in range(S // gt):
                        gcount += 1
                        if gcount > glim:
                            continue
                        vg = vgp.tile([128, KC, gt], mdt, tag="vg",
                                      name=f"vg_b{b}g{g}_{u}")
                        t0 = b * S + g * gt
                        nc.sync.dma_start(
                            vg[:], valT[:, :, t0:t0 + gt].transpose([1, 0, 2]))
                        for sub in range(gt // 128):
                            tt = g * (gt // 128) + sub
                            pr = scrp.tile([128, 2], F32, tag="pr",
                                           name=f"pr_b{b}t{tt}_{u}")
                            fslice = featbuf[:, tt * H:(tt + 1) * H]
                            for half in range(2):
                                fp = featps.tile([128, 512], F32, tag="featps",
                                                 name=f"fp_b{b}t{tt}h{half}_{u}")
                                for kc in range(KC):
                                    nc.tensor.matmul(
                                        fp[:],
                                        _mmv(vg[:, kc, sub * 128:(sub + 1) * 128]),
                                        _mmv(wv_sb[:, kc, half * 512:(half + 1) * 512]),
                                        start=(kc == 0), stop=False,
                                    )
                                # + q[b,:] + pc[s]*Wc[h] fused rank-2 (K=2):
                                # lhsT rows = [ones; pc], rhs rows = [q; Wc]
                                nc.tensor.matmul(
                                    fp[:],
                                    _mmv(qpc2_b[0:2, tt * 128:(tt + 1) * 128]),
                                    _mmv(qwc_b[0:2, half * 512:(half + 1) * 512]),
                                    start=False, stop=True,
                                )
                                # spill feat PSUM -> SBUF
                                nc.vector.tensor_copy(
                                    fslice[:, half * 512:(half + 1) * 512], fp[:])
                                th = tanhp.tile([128, 512], fdt, tag="tanh",
                                                name=f"th_b{b}t{tt}h{half}_{u}")
                                nc.scalar.activation(th[:], fp[:], AF.Tanh)
                                scr = scrp.tile([128, 512], fdt, tag="scr",
                                                name=f"scr_b{b}t{tt}h{half}_{u}")
                                nc.vector.tensor_mul(
                                    scr[:], th[:],
                                    web_sb[:, half * 512:(half + 1) * 512])
                                nc.vector.tensor_reduce(
                                    pr[:, half:half + 1], scr[:],
                                    mybir.AxisListType.X, ALU.add)
                            nc.vector.tensor_add(scores[:, tt:tt + 1],
                                                 pr[:, 0:1], pr[:, 1:2])

                    # ---- batch epilogue: softmax + context ---------------
                    if _abl("noepi"):
                        continue
                    sm = smallp.tile([128, NTT], F32, tag="sm", name=f"sm_b{b}_{u}")
                    nc.vector.tensor_add(sm[:], scores[:],
                                         mpen_sb[:, b * NTT:(b + 1) * NTT])
                    p_mat = smallp.tile([128, NTT], F32, tag="pmat",
                                        name=f"p_b{b}_{u}")
                    sumcol = smallp.tile([128, 1], F32, tag="sumcol",
                                         name=f"sumcol_b{b}_{u}")
                    nc.scalar.activation(p_mat[:], sm[:], AF.Exp)
                    nc.vector.tensor_reduce(sumcol[:], p_mat[:],
                                            mybir.AxisListType.X, ALU.add)
                    dp = spsum.tile([1, 1], F32, tag="sp", name=f"dp_b{b}_{u}")
                    nc.tensor.matmul(dp[:], ones_f[:, 0:1], sumcol[:],
                                     start=True, stop=True)
                    inv_sb = smallp.tile([1, 1], F32, tag="inv",
                                         name=f"inv_b{b}_{u}")
                    nc.vector.reciprocal(inv_sb[:], dp[:])
                    ib = spsum.tile([128, 1], F32, tag="sp", name=f"ib_b{b}_{u}")
                    nc.tensor.matmul(ib[:], ones_f[0:1, 0:128], inv_sb[:],
                                     start=True, stop=True)
                    alphas = smallp.tile([128, NTT], F32, tag="alphas",
                                         name=f"alphas_b{b}_{u}")
                    nc.vector.tensor_scalar_mul(alphas[:], p_mat[:], ib[:])
                    cov = smallp.tile([128, NTT], F32, tag="cov",
                                      name=f"cov_b{b}_{u}")
                    nc.vector.tensor_add(cov[:], alphas[:],
                                         pcT_sb[:, b * NTT:(b + 1) * NTT])
                    nc.sync.dma_start(
                        alph_out[b].rearrange("(c p) -> p c", p=128), alphas[:])
                    nc.sync.dma_start(
                        ncov_out[b].rearrange("(c p) -> p c", p=128), cov[:])

                    if _abl("noctx"):
                        continue
                    # apc[b] = sum_s alphas * pc  (DVE + f32 partition-sum)
                    am = smallp.tile([128, NTT], F32, tag="am",
                                     name=f"am_b{b}_{u}")
                    nc.vector.tensor_mul(am[:], alphas[:],
                                         pcT_sb[:, b * NTT:(b + 1) * NTT])
                    red = smallp.tile([128, 1], F32, tag="apcred",
                                      name=f"red_b{b}_{u}")
                    nc.vector.tensor_reduce(red[:], am[:],
                                            mybir.AxisListType.X, ALU.add)
                    apc_ps = spsum.tile([1, 1], F32, tag="sp",
                                        name=f"apcps_b{b}_{u}")
                    nc.tensor.matmul(apc_ps[:], ones_f[:, 0:1], red[:],
                                     start=True, stop=True)
                    nc.vector.tensor_copy(apc_row[0:1, b:b + 1], apc_ps[:])

                    a4 = smallp.tile([128, NTT, BLOC], cdt, tag="a4",
                                     name=f"a4_b{b}_{u}")
                    if cdt == F32R:
                        nc.vector.tensor_copy(
                            a4[:].rearrange("p a b -> p (a b)"), zeros_f[:])
                    else:
                        nc.vector.memset(a4[:], 0.0)
                    nc.vector.tensor_copy(a4[:, :, b], alphas[:])
                    for tt in range(NTT):
                        st = (b == 0 and tt == 0)
                        sp = (b == BLOC - 1 and tt == NTT - 1)
                        for half in range(2):
                            nc.tensor.matmul(
                                ctx_psum[:, half * 512:(half + 1) * 512],
                                _featv(a4[:, tt, :]),
                                _featv(featbuf[:, tt * H + half * 512:
                                               tt * H + (half + 1) * 512]),
                                start=st, stop=sp,
                            )

                # ---- final context correction ----------------------------
                if _abl("noctx") or _abl("noepi"):
                    zc = smallp.tile([BLOC, H], F32, tag="ctxsb", bufs=1,
                                     name=f"zc_{u}")
                    nc.vector.memset(zc[:], 0.0)
                    nc.sync.dma_start(ctx_out[:], zc[:])
                    continue
                nc.sync.dma_start(apc_sb4[:], apc_row[:])
                tmp_sb = smallp.tile([BLOC, H], F32, tag="tmp", bufs=1,
                                     name=f"tmp_{u}")
                nc.vector.tensor_sub(tmp_sb[:], ctx_psum[:], q_used[:])
                # q_sb4 is dead after the subtraction above; reuse as scratch
                nc.vector.tensor_scalar_mul(q_sb4[:], wc4n_sb[:],
                                            apc_sb4[:, 0:1])
                ctx_sb = smallp.tile([BLOC, H], F32, tag="ctxsb", bufs=1,
                                     name=f"ctxsb_{u}")
                nc.vector.tensor_add(ctx_sb[:], tmp_sb[:], q_sb4[:])
                nc.sync.dma_start(ctx_out[:], ctx_sb[:])

    nc.compile()
    return nc


# ---------------------------------------------------------------------------
# host-side input prep
# ---------------------------------------------------------------------------

def prep_in_maps(value, query, padding_mask, pre_coverage, Wv, Wq, We, Wc):
    mmnp, fnp = _mm_np(), _feat_np()
    wv_h = np.ascontiguousarray(Wv.reshape(KC, 128, H)).astype(mmnp)
    wq_h = np.ascontiguousarray(Wq.reshape(KC, 128, H)).astype(mmnp)
    web_h = np.ascontiguousarray(np.broadcast_to(We, (128, H))).astype(fnp)
    wc_row_h = Wc.reshape(1, H).astype(mmnp)
    # the context correction must subtract Wc exactly as rounded into feat
    wc_used = wc_row_h.astype(np.float32)
    if CFG["mm"] == "f32r":
        # PE reads f32r operands rounded to 11 mantissa bits (RNE), measured
        # on hardware; mirror that so the correction matches feat exactly.
        wc_used = _round_f32r(wc_used)
    wc4n_h = np.ascontiguousarray(np.broadcast_to(-wc_used, (BLOC, H)))

    in_maps = []
    for c in range(NCORES):
        sl = slice(c * BLOC, (c + 1) * BLOC)
        val_c = value[sl]                      # [4, S, 2H]
        valT = val_c.transpose(2, 0, 1).reshape(KC, 128, TOK).astype(mmnp)
        qT = query[sl].T.reshape(KC, 128, BLOC).astype(mmnp)
        pcT = np.ascontiguousarray(
            pre_coverage[sl].reshape(BLOC, NTT, 128).transpose(2, 0, 1)
        ).reshape(128, BLOC * NTT).astype(np.float32)
        mp = np.ascontiguousarray(
            padding_mask[sl].reshape(BLOC, NTT, 128).transpose(2, 0, 1)
        ).reshape(128, BLOC * NTT).astype(np.float32) * np.float32(-1e30)
        in_maps.append({
            "valT": valT, "wv": wv_h, "wq": wq_h, "qT": qT,
            "pcT": pcT, "pcTf": pcT.astype(fnp), "mpen": mp,
            "qpc2": np.concatenate(
                [np.ones((1, TOK), np.float32),
                 pre_coverage[sl].reshape(1, TOK)], axis=0).astype(mmnp),
            "wc_row": wc_row_h,
            "web": web_h, "wc4n": wc4n_h,
        })
    return in_maps


def assemble_outputs(results):
    ctx = np.stack([results[c]["ctx"] for c in range(NCORES)])    # [8,4,H]
    alph = np.stack([results[c]["alph"] for c in range(NCORES)])  # [8,4,S]
    ncov = np.stack([results[c]["ncov"] for c in range(NCORES)])
    context = ctx.reshape(B, H)[:, None, :].astype(np.float32)
    alphas = alph.reshape(B, S).astype(np.float32)
    new_coverage = ncov.reshape(B, S).astype(np.float32)
    return context, alphas, new_coverage


_CACHED_NC = None


def kernel(value, query, padding_mask, pre_coverage, Wv, Wq, We, Wc):
    global _CACHED_NC
    value = np.asarray(value)
    query = np.asarray(query)
    padding_mask = np.asarray(padding_mask)
    pre_coverage = np.asarray(pre_coverage)
    Wv, Wq = np.asarray(Wv), np.asarray(Wq)
    We, Wc = np.asarray(We), np.asarray(Wc)

    if _CACHED_NC is None:
        _CACHED_NC = build_kernel()
    nc = _CACHED_NC
    in_maps = prep_in_maps(value, query, padding_mask, pre_coverage,
                           Wv, Wq, We, Wc)
    res = bass_utils.run_bass_kernel_spmd(nc, in_maps,
                                          core_ids=list(range(NCORES)))
    return assemble_outputs(res.results)
